# revision 5
# baseline (speedup 1.0000x reference)
"""Trainium2 Bass kernel for a pre-LN transformer block (B=2, T=2048, D=1024,
NH=16, HD=64, DFF=4096) on 8 NeuronCores.

Sharding: each core owns a contiguous 512-token slab of one batch (4 cores
per batch). Zero inter-core communication: every core recomputes K/V for its
whole batch (the only cross-token coupling), then computes attention + MLP
for its own slab only. The host rotates each core's batch tokens so the
owned slab sits at rows [0:512) -> one uniform SPMD program; causality is
carried by per-core mask data.

Precision: attention-branch matmuls (qkv, scores, att@v) in bf16; residual
stream matmuls (proj, fc1, fc2) in float32r; fp32 PSUM accumulation
everywhere. LayerNorm affine params are folded into adjacent matmul weights
on the host; softmax skips max-subtraction (logits bounded ~|2.6|); the
softmax denominator comes from a ones-column appended to V.
"""

import sys

for _p in ("/opt/trn_rl_repo", "/root/.axon_site/_ro/trn_rl_repo"):
    if _p not in sys.path:
        sys.path.insert(0, _p)

import numpy as np
import ml_dtypes

import concourse.bass as bass
import concourse.tile as tile
from concourse import bacc, mybir
from concourse.bass_utils import run_bass_kernel_spmd

B = 2
T = 2048
D = 1024
NH = 16
HD = 64
DFF = 4 * D
EPS = 1e-5
P = 128
KO = D // P            # 8 contraction tiles for D
N_CORES = 8
CPB = N_CORES // B     # cores per batch
TC = T // CPB          # 512 own tokens per core
NT = T // P            # 16 token tiles per batch
NQT = TC // P          # 4 own-token tiles
FC = 512               # free-dim chunk for matmuls
NKT = T // P           # 16 key tiles
NFT = DFF // P         # 32 dff tiles

f32 = mybir.dt.float32
f32r = mybir.dt.float32r
bf16 = mybir.dt.bfloat16
AF = mybir.ActivationFunctionType
ALU = mybir.AluOpType

_CACHE = {}


def build_nc():
    nc = bacc.Bacc("TRN2", target_bir_lowering=False)

    io = {}
    io["xb"] = nc.declare_dram_parameter("xb", [T, D], f32, isOutput=False)
    io["x_own"] = nc.declare_dram_parameter("x_own", [TC, D], f32, isOutput=False)
    io["w_kv"] = nc.declare_dram_parameter("w_kv", [D, 2 * D], bf16, isOutput=False)
    io["w_q"] = nc.declare_dram_parameter("w_q", [D, D], bf16, isOutput=False)
    io["b_k"] = nc.declare_dram_parameter("b_k", [KO, P, 1], f32, isOutput=False)
    io["b_q"] = nc.declare_dram_parameter("b_q", [KO, P, 1], f32, isOutput=False)
    io["w_proj"] = nc.declare_dram_parameter("w_proj", [D, D], f32r, isOutput=False)
    io["fc1_w"] = nc.declare_dram_parameter("fc1_w", [D, DFF], f32r, isOutput=False)
    io["fc1_b"] = nc.declare_dram_parameter("fc1_b", [NFT, P, 1], f32, isOutput=False)
    io["fc2_w"] = nc.declare_dram_parameter("fc2_w", [DFF, D], f32r, isOutput=False)
    io["fc2_b_rep"] = nc.declare_dram_parameter("fc2_b_rep", [P, D], f32, isOutput=False)
    io["mask"] = nc.declare_dram_parameter("mask", [P, NKT, TC], bf16, isOutput=False)
    io["ident"] = nc.declare_dram_parameter("ident", [P, P], f32r, isOutput=False)
    io["out"] = nc.declare_dram_parameter("out", [TC, D], f32, isOutput=True)

    with tile.TileContext(nc) as tc:
        _emit(nc, tc, io)
    nc.compile()
    return nc


def _emit(nc, tc, io):
    xb, x_own, w_kv, w_q = io["xb"], io["x_own"], io["w_kv"], io["w_q"]
    b_k, b_q, w_proj = io["b_k"], io["b_q"], io["w_proj"]
    fc1_w, fc1_b, fc2_w = io["fc1_w"], io["fc1_b"], io["fc2_w"]
    fc2_b_rep, mask, out, ident_d = io["fc2_b_rep"], io["mask"], io["out"], io["ident"]

    from contextlib import ExitStack

    with ExitStack() as ctx:
        singles = ctx.enter_context(tc.tile_pool(name="singles", bufs=1))
        # two PSUM pools; shared tags keep total <= 5 banks
        psA = ctx.enter_context(tc.tile_pool(name="psA", bufs=2, space="PSUM"))
        psB = ctx.enter_context(tc.tile_pool(name="psB", bufs=3, space="PSUM"))

        ident = singles.tile([P, P], f32r)
        nc.sync.dma_start(out=ident, in_=ident_d.ap())
        eps_t = singles.tile([P, 1], f32)
        nc.vector.memset(eps_t, EPS)

        # attn_out survives phases A-D: outermost scope
        pOut = ctx.enter_context(tc.tile_pool(name="pOut", bufs=1))
        attn_out = pOut.tile([P, NQT, D], f32r)

        pABC_cm = tc.tile_pool(name="pABC", bufs=1)
        pABC = pABC_cm.__enter__()

        # ---------- Phase A: ln1 over the whole (rotated) batch ----------
        xnT = pABC.tile([P, KO, T], bf16)
        with tc.tile_pool(name="ln1", bufs=3) as ln1p:
            for t in range(NT):
                x_t = ln1p.tile([P, D], f32)
                nc.sync.dma_start(out=x_t, in_=xb.ap()[t * P:(t + 1) * P, :])
                stats = ln1p.tile([P, 2, 6], f32)
                nc.vector.bn_stats(out=stats[:, 0, :], in_=x_t[:, 0:512])
                nc.vector.bn_stats(out=stats[:, 1, :], in_=x_t[:, 512:1024])
                mv = ln1p.tile([P, 2], f32)
                nc.vector.bn_aggr(out=mv, in_=stats)
                std = ln1p.tile([P, 1], f32)
                nc.scalar.activation(out=std, in_=mv[:, 1:2], func=AF.Sqrt,
                                     bias=eps_t, scale=1.0)
                rstd = ln1p.tile([P, 1], f32)
                nc.vector.reciprocal(out=rstd, in_=std)
                xn = ln1p.tile([P, D], f32r)
                nc.vector.tensor_scalar(out=xn, in0=x_t, scalar1=mv[:, 0:1],
                                        scalar2=rstd, op0=ALU.subtract, op1=ALU.mult)
                for ko in range(KO):
                    tp = psA.tile([P, P], f32r, tag="t1")
                    nc.tensor.transpose(tp, xn[:, ko * P:(ko + 1) * P], ident)
                    nc.vector.tensor_copy(out=xnT[:, ko, t * P:(t + 1) * P], in_=tp)

        # ---------- Phase B: K^T, V rows (augmented), Q^T ----------
        kT = pABC.tile([P, KO, T], bf16)
        v_aug = pABC.tile([P, NT, NH, HD + 1], bf16)
        nc.vector.memset(v_aug[:, :, :, HD:HD + 1], 1.0)
        qT = pABC.tile([P, KO, TC], bf16)

        wv_sb = pABC.tile([P, KO, D], bf16)   # V weights resident (2 MB)
        nc.sync.dma_start(out=wv_sb,
                          in_=w_kv.ap()[:, D:2 * D].rearrange("(ko p) m -> p ko m", p=P))

        with tc.tile_pool(name="wkv", bufs=3) as wp, \
             tc.tile_pool(name="bias", bufs=2) as bp:
            for ct in range(KO):  # K^T: [128ch, T]
                wt = wp.tile([P, KO, P], bf16)
                nc.sync.dma_start(
                    out=wt,
                    in_=w_kv.ap()[:, ct * P:(ct + 1) * P]
                        .rearrange("(ko p) m -> p ko m", p=P))
                bt = bp.tile([P, 1], f32)
                nc.sync.dma_start(out=bt, in_=b_k.ap()[ct])
                for nt_ in range(T // FC):
                    acc = psB.tile([P, FC], f32, tag="t2")
                    for ko in range(KO):
                        nc.tensor.matmul(acc, wt[:, ko, :],
                                         xnT[:, ko, nt_ * FC:(nt_ + 1) * FC],
                                         start=(ko == 0), stop=(ko == KO - 1))
                    nc.scalar.activation(out=kT[:, ct, nt_ * FC:(nt_ + 1) * FC],
                                         in_=acc, func=AF.Identity, bias=bt, scale=1.0)
            for tt in range(NT):  # V rows
                for vc in range(2):
                    acc = psB.tile([P, FC], f32, tag="t2")
                    for ko in range(KO):
                        nc.tensor.matmul(acc, xnT[:, ko, tt * P:(tt + 1) * P],
                                         wv_sb[:, ko, vc * FC:(vc + 1) * FC],
                                         start=(ko == 0), stop=(ko == KO - 1))
                    nc.scalar.copy(
                        out=v_aug[:, tt, vc * 8:(vc + 1) * 8, 0:HD],
                        in_=acc.rearrange("p (h d) -> p h d", h=8))
            for ct in range(KO):  # Q^T (w_q pre-scaled by 1/sqrt(HD))
                wt3 = wp.tile([P, KO, P], bf16, tag="wq")
                nc.sync.dma_start(
                    out=wt3,
                    in_=w_q.ap()[:, ct * P:(ct + 1) * P]
                        .rearrange("(ko p) m -> p ko m", p=P))
                bt = bp.tile([P, 1], f32)
                nc.sync.dma_start(out=bt, in_=b_q.ap()[ct])
                acc = psB.tile([P, TC], f32, tag="t2")
                for ko in range(KO):
                    nc.tensor.matmul(acc, wt3[:, ko, :], xnT[:, ko, 0:TC],
                                     start=(ko == 0), stop=(ko == KO - 1))
                nc.scalar.activation(out=qT[:, ct, :], in_=acc,
                                     func=AF.Identity, bias=bt, scale=1.0)

        # ---------- Phase C: attention ----------
        mask_sb = pABC.tile([P, NKT, TC], bf16)
        nc.sync.dma_start(out=mask_sb, in_=mask.ap())

        with tc.tile_pool(name="attp", bufs=2) as attp:
            for h in range(NH):
                hp = (h * HD) % P
                hko = (h * HD) // P
                attT = attp.tile([P, NKT, TC], bf16)
                for kt in range(NKT):
                    sc = psA.tile([P, TC], f32, tag="t1")
                    nc.tensor.matmul(sc,
                                     kT[hp:hp + HD, hko, kt * P:(kt + 1) * P],
                                     qT[hp:hp + HD, hko, :],
                                     start=True, stop=True)
                    nc.scalar.activation(out=attT[:, kt, :], in_=sc, func=AF.Exp)
                nc.vector.tensor_mul(out=attT[:, :, :], in0=attT[:, :, :],
                                     in1=mask_sb[:, :, :])
                for qt in range(NQT):
                    av = psB.tile([P, HD + 1], f32, tag="t2")
                    for kt in range(NKT):
                        nc.tensor.matmul(av,
                                         attT[:, kt, qt * P:(qt + 1) * P],
                                         v_aug[:, kt, h, :],
                                         start=(kt == 0), stop=(kt == NKT - 1))
                    recip = attp.tile([P, 1], f32, tag="recip")
                    nc.vector.reciprocal(out=recip, in_=av[:, HD:HD + 1])
                    nc.vector.tensor_scalar(
                        out=attn_out[:, qt, h * HD:(h + 1) * HD],
                        in0=av[:, 0:HD], scalar1=recip, scalar2=None, op0=ALU.mult)

        pABC_cm.__exit__(None, None, None)   # xnT/kT/v/qT/mask done

        # ---------- Phase D: proj + residual -> x2; ln2 -> xn2T ----------
        pDE_cm = tc.tile_pool(name="pDE", bufs=1)
        pDE = pDE_cm.__enter__()
        pW_cm = tc.tile_pool(name="pW", bufs=1)
        pW = pW_cm.__enter__()
        x2 = pDE.tile([P, NQT, D], f32)
        xn2T = pDE.tile([P, KO, TC], f32r)
        wproj_sb = pW.tile([P, KO, D], f32r)   # 4 MB resident
        nc.sync.dma_start(out=wproj_sb,
                          in_=w_proj.ap().rearrange("(ko p) m -> p ko m", p=P))

        with tc.tile_pool(name="projp", bufs=3) as pp:
            attn_outT = pDE.tile([P, KO, TC], f32r)
            for qt in range(NQT):
                for ko in range(KO):
                    tp = psA.tile([P, P], f32r, tag="t1")
                    nc.tensor.transpose(
                        tp, attn_out[:, qt, ko * P:(ko + 1) * P], ident)
                    nc.vector.tensor_copy(
                        out=attn_outT[:, ko, qt * P:(qt + 1) * P], in_=tp)
            for qt in range(NQT):
                xo = pp.tile([P, D], f32, tag="xo")
                nc.sync.dma_start(out=xo, in_=x_own.ap()[qt * P:(qt + 1) * P, :])
                for oc in range(D // FC):
                    acc = psB.tile([P, FC], f32, tag="t2")
                    for ko in range(KO):
                        nc.tensor.matmul(acc, attn_outT[:, ko, qt * P:(qt + 1) * P],
                                         wproj_sb[:, ko, oc * FC:(oc + 1) * FC],
                                         start=(ko == 0), stop=(ko == KO - 1))
                    nc.vector.tensor_add(out=x2[:, qt, oc * FC:(oc + 1) * FC],
                                         in0=acc, in1=xo[:, oc * FC:(oc + 1) * FC])
                stats = pp.tile([P, 2, 6], f32, tag="st2")
                nc.vector.bn_stats(out=stats[:, 0, :], in_=x2[:, qt, 0:512])
                nc.vector.bn_stats(out=stats[:, 1, :], in_=x2[:, qt, 512:1024])
                mv = pp.tile([P, 2], f32, tag="mv2")
                nc.vector.bn_aggr(out=mv, in_=stats)
                std = pp.tile([P, 1], f32, tag="sd2")
                nc.scalar.activation(out=std, in_=mv[:, 1:2], func=AF.Sqrt,
                                     bias=eps_t, scale=1.0)
                rstd = pp.tile([P, 1], f32, tag="rs2")
                nc.vector.reciprocal(out=rstd, in_=std)
                xn2 = pp.tile([P, D], f32r, tag="xn2")
                nc.vector.tensor_scalar(out=xn2, in0=x2[:, qt, :], scalar1=mv[:, 0:1],
                                        scalar2=rstd, op0=ALU.subtract, op1=ALU.mult)
                for ko in range(KO):
                    tp = psA.tile([P, P], f32r, tag="t1")
                    nc.tensor.transpose(tp, xn2[:, ko * P:(ko + 1) * P], ident)
                    nc.vector.tensor_copy(
                        out=xn2T[:, ko, qt * P:(qt + 1) * P], in_=tp)

        pW_cm.__exit__(None, None, None)   # proj weights done

        # ---------- Phase E: fc1 -> gelu -> hT; fc2 + residual -> out ----------
        pE_cm = tc.tile_pool(name="pE", bufs=1)
        pE = pE_cm.__enter__()
        hT = pE.tile([P, NFT, TC], f32r)
        with tc.tile_pool(name="fc1p", bufs=3) as f1p:
            for ft in range(NFT):
                wt = f1p.tile([P, KO, P], f32r)
                nc.sync.dma_start(
                    out=wt,
                    in_=fc1_w.ap()[:, ft * P:(ft + 1) * P]
                        .rearrange("(ko p) m -> p ko m", p=P))
                bt = f1p.tile([P, 1], f32, tag="b1")
                nc.sync.dma_start(out=bt, in_=fc1_b.ap()[ft])
                acc = psB.tile([P, TC], f32, tag="t2")
                for ko in range(KO):
                    nc.tensor.matmul(acc, wt[:, ko, :], xn2T[:, ko, :],
                                     start=(ko == 0), stop=(ko == KO - 1))
                nc.scalar.activation(out=hT[:, ft, :], in_=acc, func=AF.Gelu,
                                     bias=bt, scale=1.0)

        with tc.tile_pool(name="fc2p", bufs=2) as f2p:
            b2rep = f2p.tile([P, D], f32, tag="b2")
            nc.sync.dma_start(out=b2rep, in_=fc2_b_rep.ap())
            for qt in range(NQT):
                for oc in range(D // FC):
                    acc = psB.tile([P, FC], f32, tag="t2")
                    for fg in range(4):
                        wt2 = f2p.tile([P, NFT // 4, FC], f32r, tag="w2")
                        nc.sync.dma_start(
                            out=wt2,
                            in_=fc2_w.ap()[fg * (DFF // 4):(fg + 1) * (DFF // 4),
                                           oc * FC:(oc + 1) * FC]
                                .rearrange("(ft p) m -> p ft m", p=P))
                        for fi in range(NFT // 4):
                            ft = fg * (NFT // 4) + fi
                            nc.tensor.matmul(acc, hT[:, ft, qt * P:(qt + 1) * P],
                                             wt2[:, fi, :],
                                             start=(ft == 0), stop=(ft == NFT - 1))
                    tmp = f2p.tile([P, FC], f32, tag="tmp")
                    nc.vector.tensor_add(out=tmp, in0=acc,
                                         in1=x2[:, qt, oc * FC:(oc + 1) * FC])
                    nc.vector.tensor_add(out=tmp, in0=tmp,
                                         in1=b2rep[:, oc * FC:(oc + 1) * FC])
                    nc.sync.dma_start(
                        out=out.ap()[qt * P:(qt + 1) * P, oc * FC:(oc + 1) * FC],
                        in_=tmp)
        pE_cm.__exit__(None, None, None)
        pDE_cm.__exit__(None, None, None)


def _stage_inputs(x, w_qkv, w_proj, ln1_w, ln1_b, ln2_w, ln2_b,
                  fc1_w, fc1_b, fc2_w, fc2_b):
    """Build the 8 per-core input maps (host-side sharding/folding)."""
    f = np.float32
    bf = ml_dtypes.bfloat16
    x = np.asarray(x, f)
    w_qkv = np.asarray(w_qkv, f)
    ln1_w, ln1_b = np.asarray(ln1_w, f), np.asarray(ln1_b, f)
    ln2_w, ln2_b = np.asarray(ln2_w, f), np.asarray(ln2_b, f)
    fc1_wf, fc1_bf = np.asarray(fc1_w, f), np.asarray(fc1_b, f)
    fc2_wf, fc2_bf = np.asarray(fc2_w, f), np.asarray(fc2_b, f)
    w_projf = np.asarray(w_proj, f)

    wq_f = ln1_w[:, None] * w_qkv
    bq_f = ln1_b @ w_qkv
    scale = 1.0 / np.sqrt(HD)
    w_q_h = np.ascontiguousarray(wq_f[:, 0:D] * scale).astype(bf)
    b_q_h = (bq_f[0:D] * scale).astype(f).reshape(KO, P, 1)
    w_kv_h = np.ascontiguousarray(wq_f[:, D:3 * D]).astype(bf)
    b_k_h = bq_f[D:2 * D].astype(f).reshape(KO, P, 1)
    b_v = bq_f[2 * D:3 * D]
    bvproj = b_v @ w_projf                  # folded into x_own residual

    fc1_w_h = (ln2_w[:, None] * fc1_wf).astype(f)
    fc1_b_h = (ln2_b @ fc1_wf + fc1_bf).astype(f).reshape(NFT, P, 1)
    fc2_b_rep_h = np.ascontiguousarray(np.broadcast_to(fc2_bf, (P, D))).astype(f)
    eye = np.eye(P, dtype=f)

    in_maps = []
    for c in range(N_CORES):
        b = c // CPB
        r0 = (c % CPB) * TC
        xb_c = np.ascontiguousarray(np.roll(x[b], -r0, axis=0))
        x_own_c = (x[b, r0:r0 + TC] + bvproj).astype(f)
        kidx = (np.arange(T) + r0) % T
        qidx = r0 + np.arange(TC)
        m = (kidx[:, None] <= qidx[None, :])
        mask_c = np.ascontiguousarray(
            m.reshape(NKT, P, TC).transpose(1, 0, 2)).astype(bf)
        in_maps.append({
            "xb": xb_c,
            "x_own": x_own_c,
            "w_kv": w_kv_h,
            "w_q": w_q_h,
            "b_k": b_k_h,
            "b_q": b_q_h,
            "w_proj": w_projf,
            "fc1_w": fc1_w_h,
            "fc1_b": fc1_b_h,
            "fc2_w": fc2_wf,
            "fc2_b_rep": fc2_b_rep_h,
            "mask": mask_c,
            "ident": eye,
        })
    return in_maps


def kernel(**inputs) -> np.ndarray:
    if "nc" not in _CACHE:
        _CACHE["nc"] = build_nc()
    nc = _CACHE["nc"]
    in_maps = _stage_inputs(**inputs)
    res = run_bass_kernel_spmd(nc, in_maps, list(range(N_CORES)))
    out = np.empty((B, T, D), np.float32)
    for c in range(N_CORES):
        b = c // CPB
        r0 = (c % CPB) * TC
        out[b, r0:r0 + TC] = res.results[c]["out"]
    return out
